# revision 22
# baseline (speedup 1.0000x reference)
"""Trainium2 Bass kernel for nn_LocalConnectivity (diamond-ring circular stencil).

out[i,j] = sum_{d=1..5} w_d * sum_{|di|+|dj|=d} x[(i+di)%H, (j+dj)%W]

Strategy: shard the grid 4x2 (1024 rows x 2048 cols per core, plus 5-wide
circular halos pre-padded on host). Per core the 60-tap stencil runs on the
TensorEngine as 11 banded matmuls (one per column shift dj in [-5,5]):
PSUM[m, c] += W_dj[k, m] * strip[k, c+5+dj], 9 row-windows of 118 rows.

All data is bf16 (error ~5e-3 << 2e-2 gate), halving HBM traffic vs fp32.
Loop order is dj-outer / chunk-inner so the stationary band is reused
across the 4 PSUM banks of a window; the 8-bank PSUM pool holds 2 windows
so window boundaries don't stall the PE. Output DMA rides only the
sync+gpsimd queues (scalar does copies; a store on its queue would
head-of-line block them), split so the HW DGE fans writes over many SDMA
engines. The last window drains chunk-by-chunk to shrink the tail.
"""
import numpy as np
from contextlib import ExitStack

import ml_dtypes

import concourse.bass as bass
import concourse.tile as tile
from concourse import bacc, mybir
from concourse.bass_utils import run_bass_kernel_spmd

N_CORES = 8
H = W = 4096
MAXD = 5
GRID_R, GRID_C = 4, 2                  # core grid: 4 row-shards x 2 col-shards
ROWS_PER_CORE = H // GRID_R            # 1024
COLS_PER_CORE = W // GRID_C            # 2048
IN_ROWS = ROWS_PER_CORE + 2 * MAXD     # 1034
IN_COLS = COLS_PER_CORE + 2 * MAXD     # 2058
NCOL = 512                             # matmul free dim (one PSUM bank, fp32)
NCHUNK = COLS_PER_CORE // NCOL         # 4
M_OUT = 118                            # output rows per row-window (128 - 2*MAXD)
# row windows: (out_row_start, K, M)
WINDOWS = []
_o = 0
while _o < ROWS_PER_CORE:
    _m = min(M_OUT, ROWS_PER_CORE - _o)
    WINDOWS.append((_o, _m + 2 * MAXD, _m))
    _o += _m

_CACHE = {}


def _band_weights(distance_weights: np.ndarray) -> np.ndarray:
    """w_flat [128, 11*118] bf16: w_flat[k, (dj+5)*118 + m] = K2d[k-m-5, dj]."""
    wd = np.asarray(distance_weights, dtype=np.float32)
    w = np.zeros((11, 128, M_OUT), dtype=np.float32)
    for dj in range(-MAXD, MAXD + 1):
        for di in range(-MAXD, MAXD + 1):
            d = abs(di) + abs(dj)
            if not (1 <= d <= MAXD):
                continue
            m = np.arange(M_OUT)
            k = m + MAXD + di
            ok = (k >= 0) & (k < 128)
            w[dj + MAXD, k[ok], m[ok]] = wd[d - 1]
    out = w.transpose(1, 0, 2).reshape(128, 11 * M_OUT)
    return np.ascontiguousarray(out.astype(ml_dtypes.bfloat16))


def _build():
    dtb = mybir.dt.bfloat16
    dtf = mybir.dt.float32
    nc = bacc.Bacc("TRN2", target_bir_lowering=False, debug=False,
                   num_devices=N_CORES)
    x = nc.dram_tensor("x", [IN_ROWS, IN_COLS], dtb, kind="ExternalInput").ap()
    wts = nc.dram_tensor("w", [128, 11 * M_OUT], dtb, kind="ExternalInput").ap()
    y = nc.dram_tensor("y", [ROWS_PER_CORE, COLS_PER_CORE], dtb,
                       kind="ExternalOutput").ap()

    with tile.TileContext(nc) as tc, ExitStack() as ctx:
        spool = ctx.enter_context(tc.tile_pool(name="strip",
                                               bufs=len(WINDOWS)))
        wpool = ctx.enter_context(tc.tile_pool(name="wts", bufs=1))
        opool = ctx.enter_context(tc.tile_pool(name="out", bufs=len(WINDOWS)))
        ppool = ctx.enter_context(tc.tile_pool(name="ps", bufs=8, space="PSUM"))

        # PE warm-up: a few dummy matmuls during the initial strip load so the
        # tensor engine is out of its low p-state when real work arrives.
        zpool = ctx.enter_context(tc.tile_pool(name="warm", bufs=1))
        warm_in = zpool.tile([128, NCOL], dtb, name="warm_in")
        nc.vector.memset(warm_in[:], 0.0)
        warm_ps = ppool.tile([128, NCOL], dtf, tag="ps", name="warm_ps")
        for _ in range(7):
            nc.tensor.matmul(warm_ps[:], warm_in[:, :128], warm_in[:],
                             start=True, stop=True)

        # Strip 0 split three ways + weights right behind so the first
        # window starts ASAP (reads stream at ~170 GB/s per queue).
        # Remaining strips are scheduled just-in-time across all queues.
        strips = [None] * len(WINDOWS)
        for wi, (out0, kdim, m) in enumerate(WINDOWS):
            strips[wi] = spool.tile([128, IN_COLS], dtb, tag="strip",
                                    name=f"strip{wi}")
        # Reads ride the two HW-DGE queues (sync/scalar) which fan reads
        # across all 16 SDMA engines (~200 GB/s each); gpsimd's SW-DGE queue
        # is reserved for writes, which HW-DGE queues do NOT fan.
        # Weights ride gpsimd (idle until the first output ~25us in) so the
        # three start-critical loads stream in parallel.
        wt = wpool.tile([128, 11 * M_OUT], dtb)
        nc.gpsimd.dma_start(wt[:], wts[:])
        out0, kdim, m = WINDOWS[0]
        h = kdim // 2
        nc.sync.dma_start(strips[0][:h, :], x[out0:out0 + h, :])
        nc.scalar.dma_start(strips[0][h:kdim, :], x[out0 + h:out0 + kdim, :])
        for wi in range(1, len(WINDOWS)):
            out0, kdim, m = WINDOWS[wi]
            q = nc.sync if wi % 2 == 0 else nc.scalar
            q.dma_start(strips[wi][:kdim, :], x[out0:out0 + kdim, :])

        for wi, (out0, kdim, m) in enumerate(WINDOWS):
            st = strips[wi]
            ot = opool.tile([m, COLS_PER_CORE], dtb, tag="out")
            pss = [ppool.tile([m, NCOL], dtf, tag="ps", name=f"ps{wi}_{i}")
                   for i in range(NCHUNK)]
            last = wi == len(WINDOWS) - 1
            if not last:
                # dj-outer: stationary band reused across the window's chunks.
                for j, dj in enumerate(range(-MAXD, MAXD + 1)):
                    wsl = wt[:kdim, (dj + MAXD) * M_OUT:(dj + MAXD) * M_OUT + m]
                    for cc in range(NCHUNK):
                        c0 = cc * NCOL + MAXD + dj
                        nc.tensor.matmul(
                            pss[cc][:], wsl, st[:kdim, c0:c0 + NCOL],
                            start=(j == 0), stop=(j == 10),
                        )
                for cc in range(NCHUNK):
                    dst = ot[:, cc * NCOL:(cc + 1) * NCOL]
                    if cc % 2 == 0:
                        nc.vector.tensor_copy(dst, pss[cc][:])
                    else:
                        nc.scalar.copy(dst, pss[cc][:])
                # Writes: gpsimd's software DGE fans row descriptors across
                # SDMA engines (~53 GB/s); the HW-DGE queues pin writes to a
                # single engine (~21 GB/s each) so they only take small
                # slices, and only for early windows (their drain has
                # runway; late-window slices would become the tail).
                if wi < len(WINDOWS) - 2:
                    t1 = 2 * m // 3
                    t2 = t1 + (m - t1) // 2
                    nc.gpsimd.dma_start(y[out0:out0 + t1, :], ot[:t1, :])
                    nc.sync.dma_start(y[out0 + t1:out0 + t2, :], ot[t1:t2, :])
                    nc.scalar.dma_start(y[out0 + t2:out0 + m, :], ot[t2:m, :])
                else:
                    nc.gpsimd.dma_start(y[out0:out0 + m, :], ot[:m, :])
            else:
                # Last window: chunk-outer so each chunk's copy + store can
                # drain while the remaining chunks still compute -> tiny tail.
                for cc in range(NCHUNK):
                    for j, dj in enumerate(range(-MAXD, MAXD + 1)):
                        c0 = cc * NCOL + MAXD + dj
                        nc.tensor.matmul(
                            pss[cc][:],
                            wt[:kdim, (dj + MAXD) * M_OUT:(dj + MAXD) * M_OUT + m],
                            st[:kdim, c0:c0 + NCOL],
                            start=(j == 0), stop=(j == 10),
                        )
                    dst = ot[:, cc * NCOL:(cc + 1) * NCOL]
                    if cc % 2 == 0:
                        nc.vector.tensor_copy(dst, pss[cc][:])
                    else:
                        nc.scalar.copy(dst, pss[cc][:])
                    nc.gpsimd.dma_start(
                        y[out0:out0 + m, cc * NCOL:(cc + 1) * NCOL],
                        ot[:, cc * NCOL:(cc + 1) * NCOL])
    nc.compile()
    return nc


def _prep(grid_spikes: np.ndarray, distance_weights: np.ndarray):
    """Build the per-core input maps (bf16 slabs + band weights)."""
    x = np.ascontiguousarray(grid_spikes, dtype=np.float32)
    assert x.shape == (H, W)
    w_flat = _band_weights(distance_weights)
    xb = x.astype(ml_dtypes.bfloat16)
    rows_all = np.arange(-MAXD, H + MAXD) % H
    cols_all = np.arange(-MAXD, W + MAXD) % W
    xpad = xb[np.ix_(rows_all, cols_all)]       # [H+10, W+10] circular pad
    in_maps = []
    for c in range(N_CORES):
        r, s = divmod(c, GRID_C)
        slab = xpad[r * ROWS_PER_CORE:r * ROWS_PER_CORE + IN_ROWS,
                    s * COLS_PER_CORE:s * COLS_PER_CORE + IN_COLS]
        in_maps.append({"x": np.ascontiguousarray(slab), "w": w_flat})
    return in_maps


def kernel(grid_spikes: np.ndarray, distance_weights: np.ndarray) -> np.ndarray:
    if "nc" not in _CACHE:
        _CACHE["nc"] = _build()
    nc = _CACHE["nc"]
    in_maps = _prep(grid_spikes, distance_weights)
    res = run_bass_kernel_spmd(nc, in_maps, list(range(N_CORES)))
    out = np.empty((H, W), dtype=np.float32)
    for c in range(N_CORES):
        r, s = divmod(c, GRID_C)
        out[r * ROWS_PER_CORE:(r + 1) * ROWS_PER_CORE,
            s * COLS_PER_CORE:(s + 1) * COLS_PER_CORE] = \
            res.results[c]["y"].astype(np.float32)
    return out


# revision 25
# speedup vs baseline: 1.0351x; 1.0351x over previous
"""Trainium2 Bass kernel for nn_LocalConnectivity (diamond-ring circular stencil).

out[i,j] = sum_{d=1..5} w_d * sum_{|di|+|dj|=d} x[(i+di)%H, (j+dj)%W]

Strategy: shard the grid 4x2 (1024 rows x 2048 cols per core, plus 5-wide
circular halos pre-padded on host). Per core the 60-tap stencil runs on the
TensorEngine as 11 banded matmuls (one per column shift dj in [-5,5]):
PSUM[m, c] += W_dj[k, m] * strip[k, c+5+dj], 9 row-windows of 118 rows.

All data is bf16 (error ~5e-3 << 2e-2 gate), halving HBM traffic vs fp32.
Loop order is dj-outer / chunk-inner so the stationary band is reused
across the 4 PSUM banks of a window; the 8-bank PSUM pool holds 2 windows
so window boundaries don't stall the PE. Output DMA rides only the
sync+gpsimd queues (scalar does copies; a store on its queue would
head-of-line block them), split so the HW DGE fans writes over many SDMA
engines. The last window drains chunk-by-chunk to shrink the tail.
"""
import numpy as np
from contextlib import ExitStack

import ml_dtypes

import concourse.bass as bass
import concourse.tile as tile
from concourse import bacc, mybir
from concourse.bass_utils import run_bass_kernel_spmd

N_CORES = 8
H = W = 4096
MAXD = 5
GRID_R, GRID_C = 4, 2                  # core grid: 4 row-shards x 2 col-shards
ROWS_PER_CORE = H // GRID_R            # 1024
COLS_PER_CORE = W // GRID_C            # 2048
IN_ROWS = ROWS_PER_CORE + 2 * MAXD     # 1034
IN_COLS = COLS_PER_CORE + 2 * MAXD     # 2058
NCOL = 512                             # matmul free dim (one PSUM bank, fp32)
NCHUNK = COLS_PER_CORE // NCOL         # 4
M_OUT = 118                            # output rows per row-window (128 - 2*MAXD)
# row windows: (out_row_start, K, M)
WINDOWS = []
_o = 0
while _o < ROWS_PER_CORE:
    _m = min(M_OUT, ROWS_PER_CORE - _o)
    WINDOWS.append((_o, _m + 2 * MAXD, _m))
    _o += _m

_CACHE = {}


def _band_weights(distance_weights: np.ndarray) -> np.ndarray:
    """w_flat [128, 11*118] bf16: w_flat[k, (dj+5)*118 + m] = K2d[k-m-5, dj]."""
    wd = np.asarray(distance_weights, dtype=np.float32)
    w = np.zeros((11, 128, M_OUT), dtype=np.float32)
    for dj in range(-MAXD, MAXD + 1):
        for di in range(-MAXD, MAXD + 1):
            d = abs(di) + abs(dj)
            if not (1 <= d <= MAXD):
                continue
            m = np.arange(M_OUT)
            k = m + MAXD + di
            ok = (k >= 0) & (k < 128)
            w[dj + MAXD, k[ok], m[ok]] = wd[d - 1]
    out = w.transpose(1, 0, 2).reshape(128, 11 * M_OUT)
    return np.ascontiguousarray(out.astype(ml_dtypes.bfloat16))


def _build():
    dtb = mybir.dt.bfloat16
    dtf = mybir.dt.float32
    nc = bacc.Bacc("TRN2", target_bir_lowering=False, debug=False,
                   num_devices=N_CORES)
    x = nc.dram_tensor("x", [IN_ROWS, IN_COLS], dtb, kind="ExternalInput").ap()
    wts = nc.dram_tensor("w", [128, 11 * M_OUT], dtb, kind="ExternalInput").ap()
    y = nc.dram_tensor("y", [ROWS_PER_CORE, COLS_PER_CORE], dtb,
                       kind="ExternalOutput").ap()

    with tile.TileContext(nc) as tc, ExitStack() as ctx:
        spool = ctx.enter_context(tc.tile_pool(name="strip",
                                               bufs=len(WINDOWS)))
        wpool = ctx.enter_context(tc.tile_pool(name="wts", bufs=1))
        opool = ctx.enter_context(tc.tile_pool(name="out", bufs=len(WINDOWS)))
        ppool = ctx.enter_context(tc.tile_pool(name="ps", bufs=8, space="PSUM"))

        # PE warm-up: a few dummy matmuls during the initial strip load so the
        # tensor engine is out of its low p-state when real work arrives.
        zpool = ctx.enter_context(tc.tile_pool(name="warm", bufs=1))
        warm_in = zpool.tile([128, NCOL], dtb, name="warm_in")
        nc.vector.memset(warm_in[:], 0.0)
        warm_ps = ppool.tile([128, NCOL], dtf, tag="ps", name="warm_ps")
        for _ in range(12):
            nc.tensor.matmul(warm_ps[:], warm_in[:, :128], warm_in[:],
                             start=True, stop=True)

        # Strip 0 split three ways + weights right behind so the first
        # window starts ASAP (reads stream at ~170 GB/s per queue).
        # Remaining strips are scheduled just-in-time across all queues.
        strips = [None] * len(WINDOWS)
        for wi, (out0, kdim, m) in enumerate(WINDOWS):
            strips[wi] = spool.tile([128, IN_COLS], dtb, tag="strip",
                                    name=f"strip{wi}")
        # Reads ride the two HW-DGE queues (sync/scalar) which fan reads
        # across all 16 SDMA engines (~200 GB/s each); gpsimd's SW-DGE queue
        # is reserved for writes, which HW-DGE queues do NOT fan.
        # Weights ride gpsimd (idle until the first output ~25us in) so the
        # three start-critical loads stream in parallel.
        wt = wpool.tile([128, 11 * M_OUT], dtb)
        nc.gpsimd.dma_start(wt[:], wts[:])
        out0, kdim, m = WINDOWS[0]
        h = kdim // 2
        nc.sync.dma_start(strips[0][:h, :], x[out0:out0 + h, :])
        nc.scalar.dma_start(strips[0][h:kdim, :], x[out0 + h:out0 + kdim, :])
        for wi in range(1, len(WINDOWS)):
            out0, kdim, m = WINDOWS[wi]
            q = nc.sync if wi % 2 == 0 else nc.scalar
            q.dma_start(strips[wi][:kdim, :], x[out0:out0 + kdim, :])

        for wi, (out0, kdim, m) in enumerate(WINDOWS):
            st = strips[wi]
            ot = opool.tile([m, COLS_PER_CORE], dtb, tag="out")
            pss = [ppool.tile([m, NCOL], dtf, tag="ps", name=f"ps{wi}_{i}")
                   for i in range(NCHUNK)]
            last = wi == len(WINDOWS) - 1
            if not last:
                # dj-outer: stationary band reused across the window's chunks.
                for j, dj in enumerate(range(-MAXD, MAXD + 1)):
                    wsl = wt[:kdim, (dj + MAXD) * M_OUT:(dj + MAXD) * M_OUT + m]
                    for cc in range(NCHUNK):
                        c0 = cc * NCOL + MAXD + dj
                        nc.tensor.matmul(
                            pss[cc][:], wsl, st[:kdim, c0:c0 + NCOL],
                            start=(j == 0), stop=(j == 10),
                        )
                for cc in range(NCHUNK):
                    dst = ot[:, cc * NCOL:(cc + 1) * NCOL]
                    if cc % 2 == 0:
                        nc.vector.tensor_copy(dst, pss[cc][:])
                    else:
                        nc.scalar.copy(dst, pss[cc][:])
                # Writes: gpsimd's software DGE fans row descriptors across
                # SDMA engines (~53 GB/s); the HW-DGE queues pin writes to a
                # single engine (~21 GB/s each) so they only take small
                # slices, and only for early windows (their drain has
                # runway; late-window slices would become the tail).
                t1 = 2 * m // 3
                t2 = t1 + (m - t1) // 2
                nc.gpsimd.dma_start(y[out0:out0 + t1, :], ot[:t1, :])
                nc.sync.dma_start(y[out0 + t1:out0 + t2, :], ot[t1:t2, :])
                nc.scalar.dma_start(y[out0 + t2:out0 + m, :], ot[t2:m, :])
            else:
                # Last window: chunk-outer so each chunk's copy + store can
                # drain while the remaining chunks still compute -> tiny tail.
                for cc in range(NCHUNK):
                    for j, dj in enumerate(range(-MAXD, MAXD + 1)):
                        c0 = cc * NCOL + MAXD + dj
                        nc.tensor.matmul(
                            pss[cc][:],
                            wt[:kdim, (dj + MAXD) * M_OUT:(dj + MAXD) * M_OUT + m],
                            st[:kdim, c0:c0 + NCOL],
                            start=(j == 0), stop=(j == 10),
                        )
                    dst = ot[:, cc * NCOL:(cc + 1) * NCOL]
                    if cc % 2 == 0:
                        nc.vector.tensor_copy(dst, pss[cc][:])
                    else:
                        nc.scalar.copy(dst, pss[cc][:])
                    q = (nc.gpsimd, nc.gpsimd, nc.sync, nc.scalar)[cc]
                    q.dma_start(
                        y[out0:out0 + m, cc * NCOL:(cc + 1) * NCOL],
                        ot[:, cc * NCOL:(cc + 1) * NCOL])
    nc.compile()
    return nc


def _prep(grid_spikes: np.ndarray, distance_weights: np.ndarray):
    """Build the per-core input maps (bf16 slabs + band weights)."""
    x = np.ascontiguousarray(grid_spikes, dtype=np.float32)
    assert x.shape == (H, W)
    w_flat = _band_weights(distance_weights)
    xb = x.astype(ml_dtypes.bfloat16)
    rows_all = np.arange(-MAXD, H + MAXD) % H
    cols_all = np.arange(-MAXD, W + MAXD) % W
    xpad = xb[np.ix_(rows_all, cols_all)]       # [H+10, W+10] circular pad
    in_maps = []
    for c in range(N_CORES):
        r, s = divmod(c, GRID_C)
        slab = xpad[r * ROWS_PER_CORE:r * ROWS_PER_CORE + IN_ROWS,
                    s * COLS_PER_CORE:s * COLS_PER_CORE + IN_COLS]
        in_maps.append({"x": np.ascontiguousarray(slab), "w": w_flat})
    return in_maps


def kernel(grid_spikes: np.ndarray, distance_weights: np.ndarray) -> np.ndarray:
    if "nc" not in _CACHE:
        _CACHE["nc"] = _build()
    nc = _CACHE["nc"]
    in_maps = _prep(grid_spikes, distance_weights)
    res = run_bass_kernel_spmd(nc, in_maps, list(range(N_CORES)))
    out = np.empty((H, W), dtype=np.float32)
    for c in range(N_CORES):
        r, s = divmod(c, GRID_C)
        out[r * ROWS_PER_CORE:(r + 1) * ROWS_PER_CORE,
            s * COLS_PER_CORE:(s + 1) * COLS_PER_CORE] = \
            res.results[c]["y"].astype(np.float32)
    return out
